# revision 9
# baseline (speedup 1.0000x reference)
"""Trainium2 Bass kernel for nn_CrossAttention_15418932593009.

Reference computation (fp32):
    q = (x @ wq1) @ wq2                      # (b, n, h*d), bottleneck 40
    k = silu(x @ wk1) @ wk2
    v = (x @ wv1) @ wv2
    split '(b n (h d)) -> (b (h n) d)'       # heads folded into sequence!
    sim  = q @ k.T * d**-0.5                 # (b, h*n, h*n) = (4, 8192, 8192)
    attn = softmax(sim, axis=-1)
    out  = attn @ v                          # (b, h*n, d)
    merge back -> (b, n, h*d); out @ wo + bo

Sharding: 8 cores = 4 batches x 2 query-head groups (heads 0-3 / 4-7).
Each core computes full K/V for its batch (all 8 heads) and attention for
its 4 query heads (4096 query rows x 8192 keys), then its partial
contribution of the output projection (its head group's slice of wo).
Host sums the two partials per batch and adds bo.

On-chip layout is "transposed": all SBUF activations keep the contraction
axis on partitions. Scores are computed as S^T tiles (128 keys x 512
queries), exp'd on ACT straight out of PSUM (no max subtraction: scores
are ~N(0, 0.6), softmax is shift-invariant and fp32 exp cannot overflow),
and fed to the A@V matmul which contracts keys on partitions. The softmax
denominator comes free from a ones-column appended to V (M=65 matmul).
Matmuls run in float32r (full PE rate at N>=256, ~tf32 precision); the
BIR verifier requires every fp32r matmul operand to be produced as
float32r, so DRAM inputs are declared float32r and on-chip producers
write float32r tiles.
"""

import numpy as np

HEADS = 8
D = 64
BOT = 40
B = 4
N = 1024
QS = 256
INNER = HEADS * D          # 512
GH = 4                     # query heads per core
KCH = HEADS * N // 128     # 64 key chunks of 128
QT = 512                   # query tile (matmul moving dim)
NQT = GH * N // QT         # 8 query tiles per core
NCORES = 8

_BUILT = {}


def _build():
    """Build the single-core Bass module (same NEFF for all 8 cores)."""
    import concourse.bass as bass
    import concourse.mybir as mybir
    import concourse.tile as tile
    from concourse import bacc

    dt = mybir.dt
    f32 = dt.float32
    f32r = dt.float32r
    AF = mybir.ActivationFunctionType
    PSUM = bass.MemorySpace.PSUM

    # Bacc (not plain Bass): its compile() pipeline moves/splits semaphore
    # waits (TRN2 allows at most 1 wait per instruction; fp32r matmuls
    # self-load weights so waits can't ride a separate LDWEIGHTS).
    nc = bacc.Bacc()

    # ---- DRAM I/O (per core); float32r = same bits as fp32 ----
    xT = nc.dram_tensor("xT", [QS, N], f32r, kind="ExternalInput")      # x[b].T
    wq1 = nc.dram_tensor("wq1", [QS, BOT], f32r, kind="ExternalInput")
    wk1 = nc.dram_tensor("wk1", [QS, BOT], f32r, kind="ExternalInput")
    wv1 = nc.dram_tensor("wv1", [QS, BOT], f32r, kind="ExternalInput")
    wq2g = nc.dram_tensor("wq2g", [BOT, GH * D], f32r, kind="ExternalInput")
    wk2 = nc.dram_tensor("wk2", [BOT, INNER], f32r, kind="ExternalInput")
    wv2 = nc.dram_tensor("wv2", [BOT, INNER], f32r, kind="ExternalInput")
    wog = nc.dram_tensor("wog", [GH * D, QS], f32r, kind="ExternalInput")
    out = nc.dram_tensor("out", [QS, N], f32, kind="ExternalOutput")    # partial^T

    with tile.TileContext(nc) as tc:
        with (
            tc.tile_pool(name="consts", bufs=1) as consts,
            tc.tile_pool(name="bigs", bufs=1) as bigs,
            tc.tile_pool(name="pp", bufs=3) as ppool,
            tc.tile_pool(name="small", bufs=2) as small,
            tc.tile_pool(name="mps", bufs=3, space=PSUM) as mpsum,
            tc.tile_pool(name="avps", bufs=2, space=PSUM) as avpsum,
        ):
            # ---- load inputs (one DMA per tensor: keeps consumer waits few) ----
            xT_sb = bigs.tile([128, 2, N], f32r)
            nc.sync.dma_start(xT_sb[:], xT.rearrange("(c p) n -> p c n", c=2))

            w1_sb = {}
            for name, t in (("q", wq1), ("k", wk1), ("v", wv1)):
                w = consts.tile([128, 2, BOT], f32r, name=f"w{name}1_sb")
                nc.sync.dma_start(w[:], t.rearrange("(c p) n -> p c n", c=2))
                w1_sb[name] = w
            wq2_sb = consts.tile([BOT, GH * D], f32r)
            nc.sync.dma_start(wq2_sb[:], wq2g[:])
            wk2_sb = consts.tile([BOT, INNER], f32r)
            nc.sync.dma_start(wk2_sb[:], wk2[:])
            wv2_sb = consts.tile([BOT, INNER], f32r)
            nc.sync.dma_start(wv2_sb[:], wv2[:])
            wog_sb = consts.tile([128, 2, QS], f32r)
            nc.sync.dma_start(wog_sb[:], wog.rearrange("(c p) n -> p c n", c=2))
            ones_sb = consts.tile([1, D], f32)
            nc.vector.memset(ones_sb[:], 1.0)

            # ---- bottleneck projections: bX^T = wX1^T @ x^T  (40, 1024) ----
            b_sb = {}
            for name in ("q", "k", "v"):
                ps = mpsum.tile([128, 1024], f32, tag="m")
                for s in range(2):
                    for cc in range(2):
                        nc.tensor.matmul(
                            ps[0:BOT, 512 * s : 512 * (s + 1)],
                            w1_sb[name][:, cc, :],
                            xT_sb[:, cc, 512 * s : 512 * (s + 1)],
                            start=(cc == 0),
                            stop=(cc == 1),
                        )
                bt = bigs.tile([BOT, N], f32r, name=f"b{name}_sb")
                if name == "k":
                    # silu(x) = x * sigmoid(x) (Silu table not in CoreSim)
                    sg = bigs.tile([BOT, N], f32, name="sg_sb")
                    nc.scalar.activation(sg[:], ps[0:BOT, :], AF.Sigmoid)
                    nc.vector.tensor_mul(bt[:], ps[0:BOT, :], sg[:])
                elif name == "q":
                    # fold the attention scale d**-0.5 into q
                    nc.scalar.mul(bt[:], ps[0:BOT, :], D**-0.5)
                else:
                    nc.scalar.copy(bt[:], ps[0:BOT, :])
                b_sb[name] = bt

            # ---- q^T (64, 4096): head-major columns, scaled ----
            qT_sb = bigs.tile([D, GH * N], f32r)
            for hl in range(GH):
                ps = mpsum.tile([128, 1024], f32, tag="m")
                for s in range(2):
                    nc.tensor.matmul(
                        ps[0:D, 512 * s : 512 * (s + 1)],
                        wq2_sb[:, D * hl : D * (hl + 1)],
                        b_sb["q"][:, 512 * s : 512 * (s + 1)],
                    )
                nc.scalar.copy(qT_sb[:, N * hl : N * (hl + 1)], ps[0:D, 0:N])

            # ---- k^T (64, 8192): head-major columns ----
            kT_sb = bigs.tile([D, HEADS * N], f32r)
            for hk in range(HEADS):
                ps = mpsum.tile([128, 1024], f32, tag="m")
                for s in range(2):
                    nc.tensor.matmul(
                        ps[0:D, 512 * s : 512 * (s + 1)],
                        wk2_sb[:, D * hk : D * (hk + 1)],
                        b_sb["k"][:, 512 * s : 512 * (s + 1)],
                    )
                nc.scalar.copy(kT_sb[:, N * hk : N * (hk + 1)], ps[0:D, 0:N])

            # ---- v natural (128 keys, d) per key chunk + ones column ----
            # chunk c (= 8*hk + pb) rows: keys [128c, 128c+128) of (hk, pos)
            v_sb = bigs.tile([128, KCH, D + 1], f32r)
            nc.vector.memset(v_sb.bitcast(f32)[:, :, D : D + 1], 0.0)
            nc.scalar.add(
                v_sb[:, :, D : D + 1], v_sb.bitcast(f32)[:, :, D : D + 1], 1.0
            )
            vv = v_sb.rearrange("p (h pb) e -> p pb h e", pb=8)
            for pb in range(8):
                ps = mpsum.tile([128, 1024], f32, tag="m")
                for hk in range(HEADS):
                    nc.tensor.matmul(
                        ps[:, D * hk : D * (hk + 1)],
                        b_sb["v"][:, 128 * pb : 128 * (pb + 1)],
                        wv2_sb[:, D * hk : D * (hk + 1)],
                    )
                nc.scalar.copy(
                    vv[:, pb, :, 0:D],
                    ps[:, 0:INNER].rearrange("p (h e) -> p h e", h=HEADS),
                )

            # ---- attention: per query tile, streamed over 32 key-chunk pairs ----
            oT_sb = bigs.tile([128, 2, N], f32r)  # [64*(hl%2)+d, hl//2, pos]
            for qt in range(NQT):
                hl, s = divmod(qt, 2)
                q_ap = qT_sb[:, N * hl + QT * s : N * hl + QT * (s + 1)]
                av = avpsum.tile([D + 1, QT], f32)
                prev_P = None
                for t in range(KCH // 2 + 1):
                    if t < KCH // 2:
                        ps = mpsum.tile([128, 1024], f32, tag="m")
                        for j in range(2):
                            c = 2 * t + j
                            nc.tensor.matmul(
                                ps[:, 512 * j : 512 * (j + 1)],
                                kT_sb[:, 128 * c : 128 * (c + 1)],
                                q_ap,
                            )
                        pt = ppool.tile([128, 1024], f32r, tag="P")
                        nc.scalar.activation(pt[:], ps[:], AF.Exp)
                    else:
                        pt = None
                    if t >= 1:
                        for j in range(2):
                            c = 2 * (t - 1) + j
                            nc.tensor.matmul(
                                av[:],
                                v_sb[:, c, :],
                                prev_P[:, 512 * j : 512 * (j + 1)],
                                start=(c == 0),
                                stop=(c == KCH - 1),
                            )
                    prev_P = pt
                # normalize: o^T = av[0:64] / av[64]  (per-query column).
                # Broadcast 1/l across partitions with a K=1 ones matmul (fp32).
                rq = small.tile([1, QT], f32, tag="rq")
                nc.vector.reciprocal(rq[:], av[D : D + 1, :])
                bc_ps = mpsum.tile([128, 1024], f32, tag="m")
                nc.tensor.matmul(bc_ps[0:D, 0:QT], ones_sb[:], rq[:])
                rb = small.tile([D, QT], f32, tag="rb")
                nc.scalar.copy(rb[:], bc_ps[0:D, 0:QT])
                pr, row = divmod(hl, 2)
                nc.vector.tensor_mul(
                    oT_sb[D * row : D * (row + 1), pr, QT * s : QT * (s + 1)],
                    av[0:D, :],
                    rb[:],
                )

            # ---- output projection: partial^T = wog^T @ o^T  (256, 1024) ----
            o_out = bigs.tile([128, 2, N], f32)
            for f in range(2):
                ps = mpsum.tile([128, 1024], f32, tag="m")
                for s2 in range(2):
                    for p in range(2):
                        nc.tensor.matmul(
                            ps[:, 512 * s2 : 512 * (s2 + 1)],
                            wog_sb[:, p, 128 * f : 128 * (f + 1)],
                            oT_sb[:, p, 512 * s2 : 512 * (s2 + 1)],
                            start=(p == 0),
                            stop=(p == 1),
                        )
                nc.scalar.copy(o_out[:, f, :], ps[:])
                nc.sync.dma_start(out[128 * f : 128 * (f + 1), :], o_out[:, f, :])

    nc.compile()
    return nc


def _get_nc():
    if "nc" not in _BUILT:
        _BUILT["nc"] = _build()
    return _BUILT["nc"]


def shard_inputs(x, wq1, wq2, wk1, wk2, wv1, wv2, wo, bo):
    """Full inputs -> list of 8 per-core input maps."""
    c = np.ascontiguousarray
    x = np.asarray(x, np.float32)
    in_maps = []
    for core in range(NCORES):
        b, g = divmod(core, 2)
        in_maps.append(
            {
                "xT": c(x[b].T.astype(np.float32)),
                "wq1": c(np.asarray(wq1, np.float32)),
                "wk1": c(np.asarray(wk1, np.float32)),
                "wv1": c(np.asarray(wv1, np.float32)),
                "wq2g": c(np.asarray(wq2, np.float32)[:, 256 * g : 256 * (g + 1)]),
                "wk2": c(np.asarray(wk2, np.float32)),
                "wv2": c(np.asarray(wv2, np.float32)),
                "wog": c(np.asarray(wo, np.float32)[256 * g : 256 * (g + 1), :]),
            }
        )
    return in_maps


def unshard_output(results, bo):
    """8 per-core partial^T (256, 1024) -> full (4, 1024, 256) output."""
    bo = np.asarray(bo, np.float32)
    out = np.empty((B, N, QS), np.float32)
    for b in range(B):
        acc = results[2 * b]["out"] + results[2 * b + 1]["out"]  # (256, 1024)
        out[b] = acc.T + bo
    return out


def kernel(x, wq1, wq2, wk1, wk2, wv1, wv2, wo, bo):
    from concourse.bass_utils import run_bass_kernel_spmd

    nc = _get_nc()
    in_maps = shard_inputs(x, wq1, wq2, wk1, wk2, wv1, wv2, wo, bo)
    res = run_bass_kernel_spmd(nc, in_maps, core_ids=list(range(NCORES)))
    return unshard_output(res.results, bo)


# revision 18
# speedup vs baseline: 1.0390x; 1.0390x over previous
"""Trainium2 Bass kernel for nn_CrossAttention_15418932593009.

Reference computation (fp32):
    q = (x @ wq1) @ wq2                      # (b, n, h*d), bottleneck 40
    k = silu(x @ wk1) @ wk2
    v = (x @ wv1) @ wv2
    split '(b n (h d)) -> (b (h n) d)'       # heads folded into sequence!
    sim  = q @ k.T * d**-0.5                 # (b, h*n, h*n) = (4, 8192, 8192)
    attn = softmax(sim, axis=-1)
    out  = attn @ v                          # (b, h*n, d)
    merge back -> (b, n, h*d); out @ wo + bo

Sharding: 8 cores = 4 batches x 2 query-head groups (heads 0-3 / 4-7).
Each core computes full K/V for its batch (all 8 heads) and attention for
its 4 query heads (4096 query rows x 8192 keys), then its partial
contribution of the output projection (its head group's slice of wo).
Host sums the two partials per batch and adds bo.

On-chip layout is "transposed": all SBUF activations keep the contraction
axis on partitions. Scores are computed as S^T tiles (128 keys x 512
queries), exp'd on ACT straight out of PSUM (no max subtraction: scores
are ~N(0, 0.6), softmax is shift-invariant and fp32 exp cannot overflow),
and fed to the A@V matmul which contracts keys on partitions. The softmax
denominator comes free from a ones-column appended to V (M=65 matmul).
Matmuls run in float32r (full PE rate at N>=256, ~tf32 precision); the
BIR verifier requires every fp32r matmul operand to be produced as
float32r, so DRAM inputs are declared float32r and on-chip producers
write float32r tiles.
"""

import numpy as np

HEADS = 8
D = 64
BOT = 40
B = 4
N = 1024
QS = 256
INNER = HEADS * D          # 512
GH = 4                     # query heads per core
KCH = HEADS * N // 128     # 64 key chunks of 128
QT = 512                   # query tile (matmul moving dim)
NQT = GH * N // QT         # 8 query tiles per core
NCORES = 8

_BUILT = {}


def _build():
    """Build the single-core Bass module (same NEFF for all 8 cores)."""
    import concourse.bass as bass
    import concourse.mybir as mybir
    import concourse.tile as tile
    from concourse import bacc

    dt = mybir.dt
    f32 = dt.float32
    f32r = dt.float32r
    AF = mybir.ActivationFunctionType
    PSUM = bass.MemorySpace.PSUM

    # Bacc (not plain Bass): its compile() pipeline moves/splits semaphore
    # waits (TRN2 allows at most 1 wait per instruction; fp32r matmuls
    # self-load weights so waits can't ride a separate LDWEIGHTS).
    nc = bacc.Bacc()

    # ---- DRAM I/O (per core); float32r = same bits as fp32 ----
    xT = nc.dram_tensor("xT", [QS, N], f32r, kind="ExternalInput")      # x[b].T
    wq1 = nc.dram_tensor("wq1", [QS, BOT], f32r, kind="ExternalInput")
    wk1 = nc.dram_tensor("wk1", [QS, BOT], f32r, kind="ExternalInput")
    wv1 = nc.dram_tensor("wv1", [QS, BOT], f32r, kind="ExternalInput")
    wq2g = nc.dram_tensor("wq2g", [BOT, GH * D], f32r, kind="ExternalInput")
    wk2 = nc.dram_tensor("wk2", [BOT, INNER], f32r, kind="ExternalInput")
    wv2 = nc.dram_tensor("wv2", [BOT, INNER], f32r, kind="ExternalInput")
    wog = nc.dram_tensor("wog", [GH * D, QS], f32r, kind="ExternalInput")
    out = nc.dram_tensor("out", [QS, N], f32, kind="ExternalOutput")    # partial^T

    with tile.TileContext(nc) as tc:
        with (
            tc.tile_pool(name="consts", bufs=1) as consts,
            tc.tile_pool(name="bigs", bufs=1) as bigs,
            tc.tile_pool(name="pp", bufs=3) as ppool,
            tc.tile_pool(name="small", bufs=2) as small,
            tc.tile_pool(name="mps", bufs=3, space=PSUM) as mpsum,
            tc.tile_pool(name="avps", bufs=2, space=PSUM) as avpsum,
        ):
            # ---- load inputs (one DMA per tensor: keeps consumer waits few) ----
            xT_sb = bigs.tile([128, 2, N], f32r)
            nc.sync.dma_start(xT_sb[:], xT.rearrange("(c p) n -> p c n", c=2))

            w1_sb = {}
            for name, t in (("q", wq1), ("k", wk1), ("v", wv1)):
                w = consts.tile([128, 2, BOT], f32r, name=f"w{name}1_sb")
                nc.sync.dma_start(w[:], t.rearrange("(c p) n -> p c n", c=2))
                w1_sb[name] = w
            wq2_sb = consts.tile([BOT, GH * D], f32r)
            nc.sync.dma_start(wq2_sb[:], wq2g[:])
            wk2_sb = consts.tile([BOT, INNER], f32r)
            nc.sync.dma_start(wk2_sb[:], wk2[:])
            wv2_sb = consts.tile([BOT, INNER], f32r)
            nc.sync.dma_start(wv2_sb[:], wv2[:])
            wog_sb = consts.tile([128, 2, QS], f32r)
            nc.sync.dma_start(wog_sb[:], wog.rearrange("(c p) n -> p c n", c=2))
            ones_sb = consts.tile([1, D], f32)
            nc.vector.memset(ones_sb[:], 1.0)

            # ---- bottleneck projections: bX^T = wX1^T @ x^T  (40, 1024) ----
            b_sb = {}
            for name in ("q", "k", "v"):
                ps = mpsum.tile([128, 1024], f32, tag="m")
                for s in range(2):
                    for cc in range(2):
                        nc.tensor.matmul(
                            ps[0:BOT, 512 * s : 512 * (s + 1)],
                            w1_sb[name][:, cc, :],
                            xT_sb[:, cc, 512 * s : 512 * (s + 1)],
                            start=(cc == 0),
                            stop=(cc == 1),
                        )
                bt = bigs.tile([BOT, N], f32r, name=f"b{name}_sb")
                if name == "k":
                    # silu(x) = x * sigmoid(x) (Silu table not in CoreSim)
                    sg = bigs.tile([BOT, N], f32, name="sg_sb")
                    nc.scalar.activation(sg[:], ps[0:BOT, 0:N], AF.Sigmoid)
                    nc.vector.tensor_mul(bt[:], ps[0:BOT, 0:N], sg[:])
                elif name == "q":
                    # fold the attention scale d**-0.5 into q
                    nc.vector.tensor_scalar_mul(bt[:], ps[0:BOT, 0:N], D**-0.5)
                else:
                    nc.vector.tensor_copy(bt[:], ps[0:BOT, 0:N])
                b_sb[name] = bt

            # ---- q^T (64, 4096): head-major columns, scaled ----
            qT_sb = bigs.tile([D, GH * N], f32r)
            for hl in range(GH):
                ps = mpsum.tile([128, 1024], f32, tag="m")
                for s in range(2):
                    nc.tensor.matmul(
                        ps[0:D, 512 * s : 512 * (s + 1)],
                        wq2_sb[:, D * hl : D * (hl + 1)],
                        b_sb["q"][:, 512 * s : 512 * (s + 1)],
                    )
                nc.vector.tensor_copy(qT_sb[:, N * hl : N * (hl + 1)], ps[0:D, 0:N])

            # ---- k^T (64, 8192): head-major columns ----
            kT_sb = bigs.tile([D, HEADS * N], f32r)
            for hk in range(HEADS):
                ps = mpsum.tile([128, 1024], f32, tag="m")
                for s in range(2):
                    nc.tensor.matmul(
                        ps[0:D, 512 * s : 512 * (s + 1)],
                        wk2_sb[:, D * hk : D * (hk + 1)],
                        b_sb["k"][:, 512 * s : 512 * (s + 1)],
                    )
                nc.vector.tensor_copy(kT_sb[:, N * hk : N * (hk + 1)], ps[0:D, 0:N])

            # ---- v natural (128 keys, d) per key chunk + ones column ----
            # chunk c (= 8*hk + pb) rows: keys [128c, 128c+128) of (hk, pos)
            v_sb = bigs.tile([128, KCH, D + 1], f32r)
            nc.vector.memset(v_sb.bitcast(f32)[:, :, D : D + 1], 0.0)
            nc.scalar.add(
                v_sb[:, :, D : D + 1], v_sb.bitcast(f32)[:, :, D : D + 1], 1.0
            )
            vv = v_sb.rearrange("p (h pb) e -> p pb h e", pb=8)
            for pb in range(8):
                ps = mpsum.tile([128, 1024], f32, tag="m")
                for hk in range(HEADS):
                    nc.tensor.matmul(
                        ps[:, D * hk : D * (hk + 1)],
                        b_sb["v"][:, 128 * pb : 128 * (pb + 1)],
                        wv2_sb[:, D * hk : D * (hk + 1)],
                    )
                nc.vector.tensor_copy(
                    vv[:, pb, :, 0:D],
                    ps[:, 0:INNER].rearrange("p (h e) -> p h e", h=HEADS),
                )

            # ---- attention: per query tile, streamed over key-chunk triples ----
            # (FD=1536 exp amortizes ACT's fixed per-instruction cost; 64
            #  chunks = 21 triples + 1 leftover). Each q-tile's finalize is
            #  deferred into the next q-tile's loop so the broadcast matmul
            #  never stalls the S/AV stream on PE.
            oT_sb = bigs.tile([128, 2, N], f32r)  # [64*(hl%2)+d, hl//2, pos]
            GROUPS = [(2 * g, 2) for g in range(KCH // 2)]

            def finalize(qt, av):
                # normalize: o^T = av[0:64] / av[64] (per-query column);
                # broadcast 1/l across partitions with a K=1 ones matmul.
                hl, s = divmod(qt, 2)
                rq = small.tile([1, QT], f32, tag="rq")
                nc.vector.reciprocal(rq[:], av[D : D + 1, :])
                bc_ps = mpsum.tile([128, 1024], f32, tag="m")
                nc.tensor.matmul(bc_ps[0:D, 0:QT], ones_sb[:], rq[:])
                rb = small.tile([D, QT], f32, tag="rb")
                nc.vector.tensor_copy(rb[:], bc_ps[0:D, 0:QT])
                pr, row = divmod(hl, 2)
                nc.vector.tensor_mul(
                    oT_sb[D * row : D * (row + 1), pr, QT * s : QT * (s + 1)],
                    av[0:D, :],
                    rb[:],
                )

            pending = None  # (qt, av) awaiting finalize
            for qt in range(NQT):
                hl, s = divmod(qt, 2)
                q_ap = qT_sb[:, N * hl + QT * s : N * hl + QT * (s + 1)]
                av = avpsum.tile([D + 1, QT], f32)
                prev = None  # (P tile, c0, cn) awaiting A@V
                for g in range(len(GROUPS) + 1):
                    if g < len(GROUPS):
                        c0, cn = GROUPS[g]
                        ps = mpsum.tile([128, 1024], f32, tag="m")
                        for j in range(cn):
                            nc.tensor.matmul(
                                ps[:, 512 * j : 512 * (j + 1)],
                                kT_sb[:, 128 * (c0 + j) : 128 * (c0 + j + 1)],
                                q_ap,
                            )
                        pt = ppool.tile([128, 1024], f32r, tag="P")
                        nc.scalar.activation(
                            pt[:, 0 : 512 * cn], ps[:, 0 : 512 * cn], AF.Exp
                        )
                        nxt = (pt, c0, cn)
                    else:
                        nxt = None
                    if g == 1 and pending is not None:
                        finalize(*pending)
                        pending = None
                    if prev is not None:
                        pt, c0, cn = prev
                        for j in range(cn):
                            c = c0 + j
                            nc.tensor.matmul(
                                av[:],
                                v_sb[:, c, :],
                                pt[:, 512 * j : 512 * (j + 1)],
                                start=(c == 0),
                                stop=(c == KCH - 1),
                            )
                    prev = nxt
                pending = (qt, av)
            finalize(*pending)

            # ---- output projection: partial^T = wog^T @ o^T  (256, 1024) ----
            o_out = bigs.tile([128, 2, N], f32)
            for f in range(2):
                ps = mpsum.tile([128, 1024], f32, tag="m")
                for s2 in range(2):
                    for p in range(2):
                        nc.tensor.matmul(
                            ps[:, 512 * s2 : 512 * (s2 + 1)],
                            wog_sb[:, p, 128 * f : 128 * (f + 1)],
                            oT_sb[:, p, 512 * s2 : 512 * (s2 + 1)],
                            start=(p == 0),
                            stop=(p == 1),
                        )
                nc.vector.tensor_copy(o_out[:, f, :], ps[:, 0:N])
                nc.sync.dma_start(out[128 * f : 128 * (f + 1), :], o_out[:, f, :])

    nc.compile()
    return nc


def _get_nc():
    if "nc" not in _BUILT:
        _BUILT["nc"] = _build()
    return _BUILT["nc"]


def shard_inputs(x, wq1, wq2, wk1, wk2, wv1, wv2, wo, bo):
    """Full inputs -> list of 8 per-core input maps."""
    c = np.ascontiguousarray
    x = np.asarray(x, np.float32)
    in_maps = []
    for core in range(NCORES):
        b, g = divmod(core, 2)
        in_maps.append(
            {
                "xT": c(x[b].T.astype(np.float32)),
                "wq1": c(np.asarray(wq1, np.float32)),
                "wk1": c(np.asarray(wk1, np.float32)),
                "wv1": c(np.asarray(wv1, np.float32)),
                "wq2g": c(np.asarray(wq2, np.float32)[:, 256 * g : 256 * (g + 1)]),
                "wk2": c(np.asarray(wk2, np.float32)),
                "wv2": c(np.asarray(wv2, np.float32)),
                "wog": c(np.asarray(wo, np.float32)[256 * g : 256 * (g + 1), :]),
            }
        )
    return in_maps


def unshard_output(results, bo):
    """8 per-core partial^T (256, 1024) -> full (4, 1024, 256) output."""
    bo = np.asarray(bo, np.float32)
    out = np.empty((B, N, QS), np.float32)
    for b in range(B):
        acc = results[2 * b]["out"] + results[2 * b + 1]["out"]  # (256, 1024)
        out[b] = acc.T + bo
    return out


def kernel(x, wq1, wq2, wk1, wk2, wv1, wv2, wo, bo):
    from concourse.bass_utils import run_bass_kernel_spmd

    nc = _get_nc()
    in_maps = shard_inputs(x, wq1, wq2, wk1, wk2, wv1, wv2, wo, bo)
    res = run_bass_kernel_spmd(nc, in_maps, core_ids=list(range(NCORES)))
    return unshard_output(res.results, bo)
